# revision 15
# baseline (speedup 1.0000x reference)
"""Trainium2 Bass kernel for nn_CNNQNetwork (dueling CNN Q-network).

Sharding: pure data parallel — batch 4096 split as 512 samples on each of the
8 NeuronCores; all weights replicated.

v2 design (vs baseline): activations stay [channel, spatial, batch] in SBUF,
but the GroupNorm pipeline is restructured so PSUM frees immediately and the
DVE/Scalar engines run few, large instructions:

  - Mean subtraction uses a parent-derived correction: sum_{c,s} conv(u) is a
    linear functional of the parent, computed as colsum(W)^T @ window_sum(u)
    (tiny matmuls) with window sums via a handful of full-width DVE adds.
    The correction is accumulated into the conv PSUM as a K=1 matmul BEFORE
    any stats are read, so the conv->relu chain never waits on statistics.
  - z' (centered) is evacuated PSUM->SBUF bf16 in one ScalarE instruction per
    q-chunk; variance = sum(z'^2) via DVE square + ones-matmul on TensorE.
  - rstd = exp(-0.5*ln(var+eps)) on ScalarE (both funcs in one table set),
    avoiding the banned Rsqrt and the slow DVE reciprocal.
  - feat = max(z',0) * (gamma_c * rstd_b) in ONE DVE scalar_tensor_tensor.
  - Children consume feat directly (GroupNorm is exactly invariant to the
    per-sample rstd scale; gamma is part of the reference activation).
  - Head: 58 K-slices x 4 m-tiles of N=512 matmuls, k-outer so the head
    weight stream is read exactly once; dueling algebra folded into layer 2.
"""

import numpy as np
import ml_dtypes

BF16 = ml_dtypes.bfloat16
B_TOTAL = 4096
NCORES = 8
BC = B_TOTAL // NCORES  # 512 samples per core
D = 128
EPS = 1e-5

# blocks: (name, src, kind, Hi, Wi, Ho, Wo)   kind 'h' = (1,2) kernel, 'v' = (2,1)
BLOCKS = [
    ("h1", "x2", "h", 4, 4, 4, 3),
    ("v1", "x3", "v", 4, 4, 3, 4),
    ("hh", "h1", "h", 4, 3, 4, 2),
    ("hv", "h1", "v", 4, 3, 3, 3),
    ("vh", "v1", "h", 3, 4, 3, 3),
    ("vv", "v1", "v", 3, 4, 2, 4),
]
S_OF = {n: ho * wo for (n, _, _, _, _, ho, wo) in BLOCKS}
NK = sum(S_OF.values())  # 58 K-slices of 128 for the head matmul
SMAX = 12
QN = 4  # four chunks of 128 samples

_cache = {}


def _conv_pieces(kind, Ho, Wo):
    """Bank-safe conv matmul pieces: per output row, split the s-range at
    PSUM bank boundaries (multiples of 4 fp32*128-lane slots = 2KB).
    Returns list of (i, j0, j1) with out slots s in [i*Wo+j0, i*Wo+j1)."""
    pieces = []
    for i in range(Ho):
        j0 = 0
        while j0 < Wo:
            s0 = i * Wo + j0
            # next bank boundary in s-space
            j1 = min(Wo, j0 + (4 - s0 % 4) if s0 % 4 else j0 + 4)
            pieces.append((i, j0, j1))
            j0 = j1
    return pieces


def _k1_pieces(S):
    """Bank-aligned s-ranges covering [0, S) for the K=1 mean-subtract."""
    return [(s0, min(s0 + 4, S)) for s0 in range(0, S, 4)]


def _build(loop_n=None):
    import concourse.bass as bass
    import concourse.tile as tile
    import concourse.mybir as mybir
    from concourse import bacc
    from concourse.masks import make_identity
    from contextlib import ExitStack, nullcontext

    dt = mybir.dt
    Alu = mybir.AluOpType
    Act = mybir.ActivationFunctionType

    nc = bacc.Bacc(
        "TRN2",
        target_bir_lowering=False,
        debug=False,
        enable_asserts=False,
        num_devices=NCORES,
    )

    # ---- DRAM I/O ----
    x2_d = nc.dram_tensor("x2", [32, 16, BC], dt.bfloat16, kind="ExternalInput")
    x3_d = nc.dram_tensor("x3", [32, 16, BC], dt.bfloat16, kind="ExternalInput")
    pwx_d = nc.dram_tensor("pwx", [32, 2, BC], dt.bfloat16, kind="ExternalInput")
    cw1_d = nc.dram_tensor("cw1", [32, 256], dt.bfloat16, kind="ExternalInput")
    cw_d = nc.dram_tensor("cw", [128, 8 * 128], dt.bfloat16, kind="ExternalInput")
    ncol1_d = nc.dram_tensor("ncol1", [32, 2], dt.bfloat16, kind="ExternalInput")
    ncol2_d = nc.dram_tensor("ncol2", [128, 8], dt.bfloat16, kind="ExternalInput")
    gam6_d = nc.dram_tensor("gam6", [1, 6 * 128], dt.float32, kind="ExternalInput")
    hw_d = nc.dram_tensor("hw", [NK, 128, 512], dt.bfloat16, kind="ExternalInput")
    fw_d = nc.dram_tensor("fw", [128, 16], dt.bfloat16, kind="ExternalInput")
    hb_d = nc.dram_tensor("hb", [128, 4], dt.float32, kind="ExternalInput")
    b2_d = nc.dram_tensor("b2", [4, 1], dt.float32, kind="ExternalInput")
    out_d = nc.dram_tensor("out", [BC, 4], dt.float32, kind="ExternalOutput")

    with tile.TileContext(nc) as tc, ExitStack() as ctx:
        singles = ctx.enter_context(tc.tile_pool(name="singles", bufs=1))

        # persistent SBUF tensors
        x2_sb = singles.tile([32, 16, BC], dt.bfloat16, tag="x2", name="x2")
        x3_sb = singles.tile([32, 16, BC], dt.bfloat16, tag="x3", name="x3")
        pwx_sb = singles.tile([32, 2, BC], dt.bfloat16, tag="pwx", name="pwx")
        cw1_sb = singles.tile([32, 256], dt.bfloat16, tag="cw1", name="cw1")
        cw_sb = singles.tile([128, 8 * 128], dt.bfloat16, tag="cw", name="cw")
        ncol1_sb = singles.tile([32, 2], dt.bfloat16, tag="ncol1", name="ncol1")
        ncol2_sb = singles.tile([128, 8], dt.bfloat16, tag="ncol2", name="ncol2")
        gam6_sb = singles.tile([1, 6 * 128], dt.float32, tag="gam6", name="gam6")
        fw_sb = singles.tile([128, 16], dt.bfloat16, tag="fw", name="fw")
        hb_sb = singles.tile([128, 4], dt.float32, tag="hb", name="hb")
        b2_sb = singles.tile([4, 1], dt.float32, tag="b2", name="b2")
        ident = singles.tile([128, 128], dt.float32, tag="ident", name="ident")
        ones_c = singles.tile([128, 1], dt.bfloat16, tag="ones_c", name="ones_c")
        ones_r = singles.tile([1, 128], dt.bfloat16, tag="ones_r", name="ones_r")
        eps1 = singles.tile([1, 1], dt.float32, tag="eps1", name="eps1")
        rstd_sb = singles.tile([1, 6 * BC], dt.float32, tag="rstd", name="rstd")

        nc.sync.dma_start(x2_sb[:], x2_d[:])
        nc.sync.dma_start(x3_sb[:], x3_d[:])
        nc.sync.dma_start(pwx_sb[:], pwx_d[:])
        nc.sync.dma_start(cw1_sb[:], cw1_d[:])
        nc.sync.dma_start(cw_sb[:], cw_d[:])
        nc.sync.dma_start(ncol1_sb[:], ncol1_d[:])
        nc.sync.dma_start(ncol2_sb[:], ncol2_d[:])
        nc.sync.dma_start(gam6_sb[:], gam6_d[:])
        nc.sync.dma_start(fw_sb[:], fw_d[:])
        nc.sync.dma_start(hb_sb[:], hb_d[:])
        nc.sync.dma_start(b2_sb[:], b2_d[:])
        make_identity(nc, ident[:])
        nc.vector.memset(ones_c[:], 1.0)
        nc.vector.memset(ones_r[:], 1.0)
        nc.vector.memset(eps1[:], EPS)

        feat = {}
        for name, _, _, _, _, ho, wo in BLOCKS:
            feat[name] = singles.tile(
                [128, ho * wo, BC], dt.bfloat16, tag=f"f_{name}", name=f"f_{name}"
            )

        with (tc.For_i(0, loop_n, 1) if loop_n else nullcontext()):
            with (
                tc.tile_pool(name="pw", bufs=2) as pwp,
                tc.tile_pool(name="zc", bufs=2) as zcp,
                tc.tile_pool(name="sq", bufs=2) as sqp,
                tc.tile_pool(name="crow", bufs=2) as crowp,
                tc.tile_pool(name="zq", bufs=2) as zqp,
                tc.tile_pool(name="gsb", bufs=2) as gsbp,
                tc.tile_pool(name="zring", bufs=2, space="PSUM") as zring,
                tc.tile_pool(name="gps", bufs=2, space="PSUM") as gps,
            ):
                zq_of = {}
                zc_of = {}

                def prep(bi):
                    name, srcn, kind, Hi, Wi, Ho, Wo = BLOCKS[bi]
                    S = Ho * Wo
                    first = srcn in ("x2", "x3")
                    parent = (x2_sb if srcn == "x2" else x3_sb) if first else feat[srcn]
                    gtile = gps.tile([128, BC], dt.float32, tag="G", name=f"psC_{name}")
                    psC = gtile[0:1, :]
                    if first:
                        pw = pwx_sb[:, (0 if srcn == "x2" else 1), :]
                        nc.tensor.matmul(
                            psC, ncol1_sb[:, bi : bi + 1], pw, start=True, stop=True
                        )
                    else:
                        # window sums of the parent via full-width DVE adds
                        if kind == "h":
                            nsum, stride, count = Wi, Wi, Hi  # C_j = sum_i p[i,j]
                        else:
                            nsum, stride, count = Hi, 1, Wi  # R_i = sum_j p[i,j]
                        lines = pwp.tile(
                            [128, 4, BC], dt.bfloat16, tag="lines", name=f"ln_{name}"
                        )
                        for j in range(nsum):
                            base = j * (1 if kind == "h" else Wi)
                            nc.vector.tensor_tensor(
                                lines[:, j, :], parent[:, base, :],
                                parent[:, base + stride, :], op=Alu.add,
                            )
                            for i in range(2, count):
                                nc.vector.tensor_tensor(
                                    lines[:, j, :], lines[:, j, :],
                                    parent[:, base + i * stride, :], op=Alu.add,
                                )
                        nwin = Wo if kind == "h" else Ho
                        pwin = pwp.tile(
                            [128, 2, BC], dt.bfloat16, tag="pwin", name=f"pw_{name}"
                        )
                        for t in range(2):
                            nc.vector.tensor_tensor(
                                pwin[:, t, :], lines[:, t, :], lines[:, t + 1, :],
                                op=Alu.add,
                            )
                            for u in range(2, nwin):
                                nc.vector.tensor_tensor(
                                    pwin[:, t, :], pwin[:, t, :], lines[:, t + u, :],
                                    op=Alu.add,
                                )
                        for t in range(2):
                            nc.tensor.matmul(
                                psC,
                                ncol2_sb[:, 2 * bi - 4 + t : 2 * bi - 3 + t],
                                pwin[:, t, :],
                                start=(t == 0),
                                stop=(t == 1),
                            )
                    crow = crowp.tile([1, BC], dt.bfloat16, tag="crow", name=f"cr_{name}")
                    nc.scalar.copy(crow[:], psC)
                    zq = zqp.tile([1, SMAX, BC], dt.bfloat16, tag="zq", name=f"zq_{name}")
                    nc.sync.dma_start(
                        zq[:, 0:S, :], crow[:, None, :].to_broadcast((1, S, BC))
                    )
                    zq_of[bi] = zq

                def convs(bi):
                    name, srcn, kind, Hi, Wi, Ho, Wo = BLOCKS[bi]
                    S = Ho * Wo
                    first = srcn in ("x2", "x3")
                    parent = (x2_sb if srcn == "x2" else x3_sb) if first else feat[srcn]
                    sview = parent[:].rearrange("c (i j) b -> c i j b", i=Hi)
                    zq = zq_of[bi]
                    zc = zcp.tile([128, SMAX, BC], dt.bfloat16, tag="zc", name=f"zc_{name}")
                    zc_of[bi] = zc
                    pieces = _conv_pieces(kind, Ho, Wo)
                    for q in range(QN):
                        q0 = q * 128
                        Z = zring.tile(
                            [128, SMAX, 128], dt.float32, tag="Z", name=f"Z_{name}{q}"
                        )
                        for t in range(2 if not first else 1):
                            for (i, j0, j1) in pieces:
                                s0, s1 = i * Wo + j0, i * Wo + j1
                                if first:
                                    lhsT = cw1_sb[:, bi * 128 : bi * 128 + 128]
                                    rhs = sview[:, i, j0:j1, q0 : q0 + 128]
                                else:
                                    tb = (bi - 2) * 2
                                    lhsT = cw_sb[:, (tb + t) * 128 : (tb + t + 1) * 128]
                                    rhs = (
                                        sview[:, i, j0 + t : j1 + t, q0 : q0 + 128]
                                        if kind == "h"
                                        else sview[:, i + t, j0:j1, q0 : q0 + 128]
                                    )
                                # start=True clears the WHOLE bank's has_written
                                # bits, so only the first piece touching each
                                # 4-slot bank may open it.
                                nc.tensor.matmul(
                                    Z[:, s0:s1, :],
                                    lhsT,
                                    rhs,
                                    start=(t == 0 and s0 % 4 == 0),
                                    stop=False,
                                    skip_group_check=True,
                                )
                        for (s0, s1) in _k1_pieces(S):
                            nc.tensor.matmul(
                                Z[:, s0:s1, :],
                                ones_r[:],
                                zq[:, s0:s1, q0 : q0 + 128],
                                start=False,
                                stop=True,
                                skip_group_check=True,
                            )
                        nc.scalar.copy(zc[:, 0:S, q0 : q0 + 128], Z[:, 0:S, :])

                def stats(bi):
                    name, srcn, kind, Hi, Wi, Ho, Wo = BLOCKS[bi]
                    S = Ho * Wo
                    CS = 128 * S
                    zc = zc_of[bi]
                    sq = sqp.tile([128, SMAX, BC], dt.bfloat16, tag="sq", name=f"sq_{name}")
                    nc.vector.tensor_tensor(
                        sq[:, 0:S, :], zc[:, 0:S, :], zc[:, 0:S, :], op=Alu.mult
                    )
                    sqacc = pwp.tile([128, BC], dt.bfloat16, tag="sqacc", name=f"sa_{name}")
                    nc.vector.tensor_tensor(sqacc[:], sq[:, 0, :], sq[:, 1, :], op=Alu.add)
                    for s in range(2, S):
                        nc.vector.tensor_tensor(sqacc[:], sqacc[:], sq[:, s, :], op=Alu.add)
                    gtile2 = gps.tile([128, BC], dt.float32, tag="G", name=f"psVG_{name}")
                    psV = gtile2[0:1, :]
                    nc.tensor.matmul(psV, ones_c[:], sqacc[:], start=True, stop=True)
                    lnr = crowp.tile([1, BC], dt.float32, tag="lnr", name=f"lnr_{name}")
                    nc.scalar.activation(
                        lnr[:], psV, func=Act.Ln, bias=eps1[:], scale=1.0 / CS
                    )
                    nc.scalar.activation(
                        rstd_sb[:, bi * BC : (bi + 1) * BC], lnr[:], func=Act.Exp,
                        scale=-0.5,
                    )
                    # G[c,b] = gamma_c * rstd_b  (outer-product matmul, fp32)
                    nc.tensor.matmul(
                        gtile2[:],
                        gam6_sb[:, bi * 128 : (bi + 1) * 128],
                        rstd_sb[:, bi * BC : (bi + 1) * BC],
                        start=True,
                        stop=True,
                        skip_group_check=True,
                    )
                    gsb = gsbp.tile([128, BC], dt.bfloat16, tag="gsb", name=f"g_{name}")
                    nc.scalar.copy(gsb[:], gtile2[:])
                    # feat = max(z',0) * G   (one DVE pass)
                    nc.vector.scalar_tensor_tensor(
                        feat[name][:],
                        zc[:, 0:S, :],
                        0.0,
                        gsb[:, None, :].to_broadcast((128, S, BC)),
                        op0=Alu.max,
                        op1=Alu.mult,
                    )

                # pipelined schedule: children's correction prep is emitted
                # right after the parent's feat (ahead in the DVE FIFO), and
                # the next block's convs are queued on TensorE before this
                # block's stats matmuls so the PE never waits on the DVE.
                prep(0)
                prep(1)
                convs(0)
                convs(1)
                stats(0)
                prep(2)
                prep(3)
                convs(2)
                stats(1)
                prep(4)
                prep(5)
                convs(3)
                stats(2)
                convs(4)
                stats(3)
                convs(5)
                stats(4)
                stats(5)

            # ---- heads ----
            with (
                tc.tile_pool(name="hwp", bufs=8) as hwp,
                tc.tile_pool(name="hidp", bufs=1) as hidp,
                tc.tile_pool(name="hs", bufs=1) as hsp,
                tc.tile_pool(name="hp", bufs=1, space="PSUM") as hp,
                tc.tile_pool(name="fp", bufs=1, space="PSUM") as fp,
                tc.tile_pool(name="tp", bufs=2, space="PSUM") as tp,
            ):
                psH = [
                    hp.tile([128, BC], dt.float32, tag=f"psH{mt}", name=f"psH{mt}")
                    for mt in range(4)
                ]
                k = 0
                for name, _, _, _, _, ho, wo in BLOCKS:
                    for s in range(ho * wo):
                        hwt = hwp.tile([128, 512], dt.bfloat16, tag="hwt", name=f"hw{k}")
                        nc.sync.dma_start(hwt[:], hw_d[k])
                        for mt in range(4):
                            nc.tensor.matmul(
                                psH[mt][:],
                                hwt[:, mt * 128 : (mt + 1) * 128],
                                feat[name][:, s, :],
                                start=(k == 0),
                                stop=(k == NK - 1),
                            )
                        k += 1
                hids = []
                for mt in range(4):
                    hid = hidp.tile([128, BC], dt.bfloat16, tag=f"hid{mt}", name=f"hid{mt}")
                    nc.scalar.activation(
                        hid[:], psH[mt][:], func=Act.Relu,
                        bias=hb_sb[:, mt : mt + 1], scale=1.0,
                    )
                    hids.append(hid)
                psF = fp.tile([4, BC], dt.float32, tag="psF", name="psF")
                for mt in range(4):
                    nc.tensor.matmul(
                        psF[:],
                        fw_sb[:, mt * 4 : (mt + 1) * 4],
                        hids[mt][:],
                        start=(mt == 0),
                        stop=(mt == 3),
                    )
                finf = hsp.tile([4, BC], dt.float32, tag="finf", name="finf")
                nc.scalar.activation(
                    finf[:], psF[:], func=Act.Identity, bias=b2_sb[:, 0:1], scale=1.0
                )
                osb = hsp.tile([128, 4, 4], dt.float32, tag="osb", name="osb")
                for qq in range(4):
                    psT = tp.tile([128, 4], dt.float32, tag="psT", name="psT")
                    nc.tensor.transpose(
                        psT[:], finf[:, qq * 128 : (qq + 1) * 128], ident[0:4, 0:4]
                    )
                    nc.scalar.copy(osb[:, qq, :], psT[:])
                nc.sync.dma_start(out_d[:].rearrange("(q p) j -> p q j", p=128), osb[:])

    nc.compile()
    return nc


def _prep_weights(inp):
    """Host-side weight preprocessing shared by all cores."""
    f32 = np.float32
    for k in ("b_h1", "b_v1", "b_hh", "b_hv", "b_vh", "b_vv"):
        assert np.allclose(inp[k], 0.0), f"conv bias {k} must be zero"
    for k in ("gb_h1", "gb_v1", "gb_hh", "gb_hv", "gb_vh", "gb_vv"):
        assert np.allclose(inp[k], 0.0), f"groupnorm beta {k} must be zero"
    gammas = {n: np.asarray(inp[f"gw_{n}"], f32) for n in S_OF}

    # first-level conv lhsT (taps stacked into K=32)
    w_h1 = np.asarray(inp["w_h1"], f32)
    w_v1 = np.asarray(inp["w_v1"], f32)
    cw1 = np.zeros((32, 256), f32)
    cw1[0:16, 0:128] = w_h1[:, :, 0, 0].T
    cw1[16:32, 0:128] = w_h1[:, :, 0, 1].T
    cw1[0:16, 128:256] = w_v1[:, :, 0, 0].T
    cw1[16:32, 128:256] = w_v1[:, :, 1, 0].T

    # second-level conv lhsT, RAW weights (children consume feat directly)
    cw = np.zeros((128, 8 * 128), f32)
    second = [("hh", "w_hh", "h"), ("hv", "w_hv", "v"),
              ("vh", "w_vh", "h"), ("vv", "w_vv", "v")]
    for idx, (name, wk, kind) in enumerate(second):
        w = np.asarray(inp[wk], f32)
        for t in range(2):
            tap = w[:, :, 0, t] if kind == "h" else w[:, :, t, 0]
            cw[:, (2 * idx + t) * 128 : (2 * idx + t + 1) * 128] = tap.T

    # correction column vectors: -(1/CS) * colsum of conv lhsT
    ncol1 = np.zeros((32, 2), f32)
    ncol1[:, 0] = -cw1[:, 0:128].sum(axis=1) / (128.0 * 12)
    ncol1[:, 1] = -cw1[:, 128:256].sum(axis=1) / (128.0 * 12)
    ncol2 = np.zeros((128, 8), f32)
    cs2 = {"hh": 128 * 8, "hv": 128 * 9, "vh": 128 * 9, "vv": 128 * 8}
    for idx, (name, _, _) in enumerate(second):
        for t in range(2):
            col = 2 * idx + t
            ncol2[:, col] = -cw[:, col * 128 : (col + 1) * 128].sum(axis=1) / cs2[name]

    gam6 = np.zeros((1, 6 * 128), f32)
    for bi, (name, *_rest) in enumerate(BLOCKS):
        gam6[0, bi * 128 : (bi + 1) * 128] = gammas[name]

    # head weights: W1c = [vw1; aw1] (512, 7424), re-tiled [kslice, c, mt*128]
    W1c = np.concatenate(
        [np.asarray(inp["vw1"], f32), np.asarray(inp["aw1"], f32)], axis=0
    )
    hw = np.empty((NK, 128, 512), f32)
    off = 0
    kidx = 0
    for name, _, _, _, _, ho, wo in BLOCKS:
        S = ho * wo
        Wb = W1c[:, off : off + 128 * S].reshape(512, 128, S)
        off += 128 * S
        for s in range(S):
            hw[kidx] = Wb[:, :, s].T  # [c, 512hidden]
            kidx += 1

    # final layer with dueling algebra folded in
    vw2 = np.asarray(inp["vw2"], f32)
    aw2 = np.asarray(inp["aw2"], f32)
    W2c = np.zeros((4, 512), f32)
    W2c[:, 0:256] = vw2[0][None, :]
    W2c[:, 256:512] = aw2 - aw2.mean(axis=0, keepdims=True)
    W2cT = W2c.T
    fw = np.zeros((128, 16), f32)
    for kt in range(4):
        fw[:, kt * 4 : (kt + 1) * 4] = W2cT[kt * 128 : (kt + 1) * 128, :]
    b2 = (
        np.asarray(inp["vb2"], f32)[0]
        + np.asarray(inp["ab2"], f32)
        - np.asarray(inp["ab2"], f32).mean()
    ).reshape(4, 1)
    hb = np.concatenate(
        [np.asarray(inp["vb1"], f32), np.asarray(inp["ab1"], f32)]
    ).reshape(4, 128).T.copy()

    return {
        "cw1": cw1.astype(BF16),
        "cw": cw.astype(BF16),
        "ncol1": ncol1.astype(BF16),
        "ncol2": ncol2.astype(BF16),
        "gam6": gam6.astype(np.float32),
        "hw": hw.astype(BF16),
        "fw": fw.astype(BF16),
        "hb": hb.astype(np.float32),
        "b2": b2.astype(np.float32),
    }


def _prep_x(xs):
    """Per-core input prep: tap-stacked [c,s,b] bf16 arrays + window sums."""
    f32 = np.float32
    n = xs.shape[0]
    x2 = np.zeros((n, 32, 4, 4), f32)
    x2[:, 0:16] = xs
    x2[:, 16:32, :, 0:3] = xs[:, :, :, 1:4]
    x3 = np.zeros((n, 32, 4, 4), f32)
    x3[:, 0:16] = xs
    x3[:, 16:32, 0:3, :] = xs[:, :, 1:4, :]
    x2 = x2.transpose(1, 2, 3, 0).reshape(32, 16, n)
    x3 = x3.transpose(1, 2, 3, 0).reshape(32, 16, n)
    # window sums over the output grids (h1: j<=2 of 4x4; v1: i<=2)
    x2b = x2.astype(BF16).astype(f32).reshape(32, 4, 4, n)
    x3b = x3.astype(BF16).astype(f32).reshape(32, 4, 4, n)
    pwx = np.zeros((32, 2, n), f32)
    pwx[:, 0] = x2b[:, :, 0:3, :].sum(axis=(1, 2))
    pwx[:, 1] = x3b[:, 0:3, :, :].sum(axis=(1, 2))
    return x2.astype(BF16), x3.astype(BF16), pwx.astype(BF16)


def _prep_x_map(xs, w):
    """Per-core input map: shared weights + this core's prepped x views."""
    x2, x3, pwx = _prep_x(xs)
    m = dict(w)
    m["x2"] = x2
    m["x3"] = x3
    m["pwx"] = pwx
    return m


def _get_nc():
    if "nc" not in _cache:
        _cache["nc"] = _build()
    return _cache["nc"]


def kernel(**inputs) -> np.ndarray:
    from concourse.bass_utils import run_bass_kernel_spmd

    nc = _get_nc()
    x = np.asarray(inputs["x"], np.float32)
    w = _prep_weights(inputs)

    in_maps = []
    for c in range(NCORES):
        xs = x[c * BC : (c + 1) * BC]
        in_maps.append(_prep_x_map(xs, w))

    res = run_bass_kernel_spmd(nc, in_maps, core_ids=list(range(NCORES)))
    out = np.concatenate([r["out"] for r in res.results], axis=0)
    return out.astype(np.float32)
